# revision 7
# baseline (speedup 1.0000x reference)
"""Builder + host glue for the ViT attention kernel on 8 trn2 cores.

Reference computation (per batch b):
    qkv = x @ w_qkv.T ; q,k,v split; per head: softmax(q k^T / sqrt(dh)) v
    out = attn @ w_out.T + b_out

Sharding: data-parallel over batch (8 batches per core).

Structure (v3):
  - Input DMAs prioritized: x lands first (split over two queues), then
    wqk/wv on a third, wo/bias last -- the QK projection saturates the PE
    as early as the data allows.
  - Double-bank PSUM tiles [128, 1024]: QK^T writes both heads' scores into
    one tile (cols 0:394 / 512:906) so a single 906-wide exp activation
    serves the whole head pair; QK-projection chunks pair up the same way
    so one strided CAST copies two chunks at once.
  - V tiles store per head [ones(64) | v(64)] so the AV matmul produces the
    softmax denominator replicated across 64 partitions at base 0: the
    normalize chain is one wide reciprocal straight from PSUM plus two
    muls -- no s-row copy, no gpsimd partition broadcast.
  - Attention pairs software-pipelined with vproj/outproj units as PE
    fillers; the final batch-pair's out-projection is split into
    half-column units so most of it overlaps the last batch and the PE
    stays warm through the tail.
  - Output stored bf16 to halve the final DMA drain.
"""

import numpy as np
import ml_dtypes

import concourse.bass as bass
import concourse.tile as tile
from concourse import bacc, mybir
from concourse.bass_utils import run_bass_kernel_spmd

P = 128
B, N, D = 64, 197, 768
H, DH = 12, 64
NCORES = 8
BPC = B // NCORES          # 8 batches per core
T = BPC * N                # 1576 tokens per core
KT = D // P                # 6 contraction tiles
NPAIR = H // 2             # 6 head pairs
SCALE = DH ** -0.5
N2 = 2 * N                 # 394
JT1 = N - P                # 69: second j-tile size

BF = mybir.dt.bfloat16
F32 = mybir.dt.float32
EXP = mybir.ActivationFunctionType.Exp
IDN = mybir.ActivationFunctionType.Identity

T_CHUNKS = [(0, 394), (394, 394), (788, 394), (1182, 394)]


def build_nc():
    nc = bacc.Bacc(
        "TRN2", target_bir_lowering=False, debug=False, num_devices=NCORES
    )
    xT = nc.dram_tensor("xT", [D, T], BF, kind="ExternalInput").ap()
    wqkT = nc.dram_tensor("wqkT", [D, 2 * D], BF, kind="ExternalInput").ap()
    wvT = nc.dram_tensor("wvT", [D, D], BF, kind="ExternalInput").ap()
    woT = nc.dram_tensor("woT", [D, D], BF, kind="ExternalInput").ap()
    bias = nc.dram_tensor("bias", [P, KT], F32, kind="ExternalInput").ap()
    outT = nc.dram_tensor("outT", [D, T], BF, kind="ExternalOutput").ap()

    with tile.TileContext(nc) as tc:
        with (
            tc.tile_pool(name="big", bufs=1) as big,
            tc.tile_pool(name="exp", bufs=8) as sb_exp,
            tc.tile_pool(name="rb", bufs=4) as sb_rb,
            tc.tile_pool(name="osb", bufs=3) as sb_osb,
            tc.tile_pool(name="ps_big", bufs=3, space="PSUM") as ps_big,
            tc.tile_pool(name="ps_o", bufs=2, space="PSUM") as ps_o,
        ):
            # ---- ACT table warmup: make exp the first ACT op ------------
            w1 = big.tile([1, 16], F32, tag="w1")
            nc.vector.memset(w1[:], 0.0)
            w2 = big.tile([1, 16], BF, tag="w2")
            nc.scalar.activation(w2[:], w1[:], EXP)

            # ---- persistent buffers + input DMAs ------------------------
            x_sb = [big.tile([P, T], BF, tag=f"x{k}", name=f"x{k}") for k in range(KT)]
            wqk_sb = [big.tile([P, 2 * D], BF, tag=f"wqk{k}", name=f"wqk{k}") for k in range(KT)]
            wv_sb = [big.tile([P, D], BF, tag=f"wv{k}", name=f"wv{k}") for k in range(KT)]
            wo_sb = [big.tile([P, D], BF, tag=f"wo{k}", name=f"wo{k}") for k in range(KT)]
            bias_sb = big.tile([P, KT], F32, tag="bias")
            # v tiles per (batch, j-tile): per head g a [ones(64) | v(64)]
            # column block so AV emits denominators at psum partitions 0:64.
            v_sb = [big.tile([P, H * P], BF, tag=f"v{i}", name=f"v{i}") for i in range(2 * BPC)]

            # x first (halves on two queues); wqk in col-pieces interleaved
            # with wv on gpsimd (finer dependency granularity); wo/bias last.
            for half in range(2):
                h0 = 788 * half
                for k in range(KT):
                    eng = nc.sync if k % 2 == 0 else nc.scalar
                    eng.dma_start(
                        x_sb[k][:, h0 : h0 + 788],
                        xT[k * P : (k + 1) * P, h0 : h0 + 788],
                    )
            def dma_wqk(piece):
                c0 = 512 * piece
                for k in range(KT):
                    nc.gpsimd.dma_start(
                        wqk_sb[k][:, c0 : c0 + 512],
                        wqkT[k * P : (k + 1) * P, c0 : c0 + 512],
                    )
            def ones_memset(i):
                ones_cols = v_sb[i][:].rearrange("p (g c) -> p g c", c=P)[
                    :, :, 0:DH
                ]
                nc.gpsimd.memset(ones_cols, 1.0)
            dma_wqk(0)
            ones_memset(0)
            ones_memset(1)
            for k in range(KT):
                nc.gpsimd.dma_start(wv_sb[k][:], wvT[k * P : (k + 1) * P, :])
            dma_wqk(1)
            dma_wqk(2)
            for i in range(2, 2 * BPC):
                ones_memset(i)
            for k in range(KT):
                nc.scalar.dma_start(wo_sb[k][:], woT[k * P : (k + 1) * P, :])
            nc.sync.dma_start(bias_sb[:], bias)

            # qk_sb[m]: m<6 -> q head-pair m ; m>=6 -> k head-pair m-6.
            # layout [e within pair (2 heads x 64), t global]
            qk_sb = [big.tile([P, T], BF, tag=f"qk{m}", name=f"qk{m}") for m in range(2 * NPAIR)]
            # attention output, [e, t] layout, tiles per (pair, batch-pair)
            at_sb = [
                [big.tile([P, N2], BF, tag=f"at{p}_{q}", name=f"at{p}_{q}") for q in range(BPC // 2)]
                for p in range(NPAIR)
            ]

            # ---- QK projection unit: (m, chunk-pair) double-bank psum ---
            def unit_m(m, cp):
                def emit():
                    ps = ps_big.tile([P, 1024], F32, tag="pb", name="pb")
                    for ci, c in enumerate((2 * cp, 2 * cp + 1)):
                        t0, tl = T_CHUNKS[c]
                        sub = ps[:, 512 * ci : 512 * ci + tl]
                        for k in range(KT):
                            nc.tensor.matmul(
                                sub,
                                wqk_sb[k][:, m * P : (m + 1) * P],
                                x_sb[k][:, t0 : t0 + tl],
                                start=(k == 0),
                                stop=(k == KT - 1),
                            )
                    src = ps[:].rearrange("p (two q) -> p two q", two=2)[
                        :, :, 0:394
                    ]
                    dst = qk_sb[m][:, 788 * cp : 788 * cp + 788].rearrange(
                        "p (two q) -> p two q", two=2
                    )
                    nc.vector.tensor_copy(out=dst, in_=src)

                return emit

            # ---- V projection unit: one (batch, j-tile) -----------------
            def unit_v(b, jt):
                def emit():
                    r0 = b * N + jt * P
                    rl = P if jt == 0 else JT1
                    i = 2 * b + jt
                    dst = v_sb[i][:rl].rearrange("p (g c) -> p g c", c=P)
                    ps = ps_big.tile([P, 1024], F32, tag="pb", name="pbv")
                    psA = ps[:rl, 0:512]
                    for k in range(KT):
                        nc.tensor.matmul(
                            psA,
                            x_sb[k][:, r0 : r0 + rl],
                            wv_sb[k][:, 0:512],
                            start=(k == 0),
                            stop=(k == KT - 1),
                        )
                    nc.scalar.copy(
                        out=dst[:, 0:8, DH:P],
                        in_=psA.rearrange("p (g c) -> p g c", c=DH),
                    )
                    psB = ps[:rl, 512:768]
                    for k in range(KT):
                        nc.tensor.matmul(
                            psB,
                            x_sb[k][:, r0 : r0 + rl],
                            wv_sb[k][:, 512:768],
                            start=(k == 0),
                            stop=(k == KT - 1),
                        )
                    nc.scalar.copy(
                        out=dst[:, 8:12, DH:P],
                        in_=psB.rearrange("p (g c) -> p g c", c=DH),
                    )

                return emit

            # ---- out-projection unit (optionally a column half) ---------
            def unit_o(q, m, half=None):
                def emit():
                    if half is None:
                        c0, cl = 0, N2
                    else:
                        c0, cl = (0, N) if half == 0 else (N, N - 1)
                        cl = N if half == 0 else N2 - N
                    ps = ps_o.tile([P, 512], F32, tag="po", name="po_o")[
                        :, 0 : cl
                    ]
                    for pk in range(NPAIR):
                        nc.tensor.matmul(
                            ps,
                            wo_sb[pk][:, m * P : (m + 1) * P],
                            at_sb[pk][q][:, c0 : c0 + cl],
                            start=(pk == 0),
                            stop=(pk == NPAIR - 1),
                        )
                    osb = sb_osb.tile([P, N2], BF, tag="osb", name="osb")[
                        :, 0:cl
                    ]
                    nc.scalar.activation(
                        osb, ps, IDN, bias=bias_sb[:, m : m + 1]
                    )
                    nc.sync.dma_start(
                        outT[
                            m * P : (m + 1) * P,
                            q * N2 + c0 : q * N2 + c0 + cl,
                        ],
                        osb,
                    )

                return emit

            # ---- attention pair, split into QK^T+exp and AV+normalize ---
            pair_state = {}

            def pair_qk(b, p):
                def emit():
                    tb = b * N
                    qT, kT = qk_sb[p], qk_sb[NPAIR + p]
                    ps = ps_big.tile([P, 1024], F32, tag="pb", name="pbs")
                    for h in (0, 1):
                        e0 = DH * h
                        sub = ps[:, 512 * h : 512 * h + N2]
                        nc.tensor.matmul(
                            sub[0:P, 0:N],
                            kT[e0 : e0 + DH, tb : tb + P],
                            qT[e0 : e0 + DH, tb : tb + N],
                            start=True,
                            stop=True,
                            tile_position=(e0, 0),
                        )
                        nc.tensor.matmul(
                            sub[0:JT1, N:N2],
                            kT[e0 : e0 + DH, tb + P : tb + N],
                            qT[e0 : e0 + DH, tb : tb + N],
                            start=True,
                            stop=True,
                            tile_position=(e0, 0),
                        )
                    es = sb_exp.tile([P, 906], BF, tag="expT", name="expT")
                    nc.scalar.activation(es[:], ps[:, 0:906], EXP)
                    pair_state[(b, p)] = es

                return emit

            def pair_av(b, p):
                def emit():
                    es = pair_state.pop((b, p))
                    pso = ps_o.tile([P, 512], F32, tag="po", name="po_a")[
                        :, :N2
                    ]
                    for h in (0, 1):
                        g = 2 * p + h
                        o0 = 512 * h
                        nc.tensor.matmul(
                            pso[:, N * h : N * h + N],
                            v_sb[2 * b][0:P, g * P : (g + 1) * P],
                            es[0:P, o0 : o0 + N],
                            start=True,
                            stop=False,
                        )
                        nc.tensor.matmul(
                            pso[:, N * h : N * h + N],
                            v_sb[2 * b + 1][0:JT1, g * P : (g + 1) * P],
                            es[0:JT1, o0 + N : o0 + N2],
                            start=False,
                            stop=True,
                        )
                    rb = sb_rb.tile([DH, N2], F32, tag="rb", name="rb")
                    nc.vector.reciprocal_approx_fast(out=rb[:], in_=pso[0:DH, :])
                    q2, c0 = b // 2, N * (b % 2)
                    for h in (0, 1):
                        nc.vector.tensor_mul(
                            out=at_sb[p][q2][
                                DH * h : DH * h + DH, c0 : c0 + N
                            ],
                            in0=pso[DH:P, N * h : N * h + N],
                            in1=rb[:, N * h : N * h + N],
                        )

                return emit

            # ---- driver -------------------------------------------------
            from collections import deque

            for cp in range(2):
                unit_m(0, cp)()
            for cp in range(2):
                unit_m(NPAIR, cp)()

            # batch 0: qk-projection m-units are the fillers; v(0) units
            # land after the first QK^T so the wv DMA has time to arrive.
            pair_qk(0, 0)()
            for p in range(1, NPAIR):
                for cp in range(2):
                    unit_m(p, cp)()
                if p == 1:
                    unit_v(0, 0)()
                    unit_v(0, 1)()
                else:
                    pair_av(0, p - 2)()
                for cp in range(2):
                    unit_m(NPAIR + p, cp)()
                pair_qk(0, p)()
            unit_v(1, 0)()
            unit_v(1, 1)()
            pair_av(0, NPAIR - 2)()
            pair_av(0, NPAIR - 1)()

            # batches 1..7: v-proj and out-proj units as fillers
            fills = {
                1: [unit_v(2, 0), unit_v(2, 1)],
                2: [unit_v(3, 0), unit_v(3, 1),
                    unit_o(0, 0), unit_o(0, 1), unit_o(0, 2), unit_o(0, 3)],
                3: [unit_v(4, 0), unit_v(4, 1), unit_o(0, 4), unit_o(0, 5)],
                4: [unit_v(5, 0), unit_v(5, 1),
                    unit_o(1, 0), unit_o(1, 1), unit_o(1, 2), unit_o(1, 3)],
                5: [unit_v(6, 0), unit_v(6, 1), unit_o(1, 4), unit_o(1, 5)],
                6: [unit_v(7, 0), unit_v(7, 1),
                    unit_o(2, 0), unit_o(2, 1), unit_o(2, 2), unit_o(2, 3)],
                7: [unit_o(3, m, half=0) for m in range(KT)]
                   + [unit_o(2, 4), unit_o(2, 5)],
            }
            for b in range(1, BPC):
                fill = deque(fills[b])
                pair_qk(b, 0)()
                for p in range(1, NPAIR):
                    if fill:
                        fill.popleft()()
                    pair_av(b, p - 1)()
                    pair_qk(b, p)()
                while fill:
                    fill.popleft()()
                pair_av(b, NPAIR - 1)()
            for m in range(KT):
                unit_o(3, m, half=1)()

    nc.compile()
    return nc


def host_in_maps(x, w_qkv, w_out, b_out):
    """Full fp32 inputs -> list of 8 per-core input dicts (bf16)."""
    bf16 = ml_dtypes.bfloat16
    wq = w_qkv[0:D] * SCALE
    wk = w_qkv[D : 2 * D]
    wv = w_qkv[2 * D : 3 * D]
    wqkT = np.ascontiguousarray(np.concatenate([wq, wk], axis=0).T).astype(bf16)
    wvT = np.ascontiguousarray(wv.T).astype(bf16)
    woT = np.ascontiguousarray(w_out.T).astype(bf16)
    bias = np.ascontiguousarray(b_out.reshape(KT, P).T).astype(np.float32)
    in_maps = []
    for c in range(NCORES):
        xc = x[c * BPC : (c + 1) * BPC].reshape(T, D)
        xT = np.ascontiguousarray(xc.T).astype(bf16)
        in_maps.append(
            {"xT": xT, "wqkT": wqkT, "wvT": wvT, "woT": woT, "bias": bias}
        )
    return in_maps


def host_gather(results):
    """8 per-core {outT: [768, 1576] bf16} -> full [64, 197, 768] fp32."""
    out = np.empty((B, N, D), dtype=np.float32)
    for c in range(NCORES):
        oc = results[c]["outT"].astype(np.float32)  # [D, T]
        out[c * BPC : (c + 1) * BPC] = oc.T.reshape(BPC, N, D)
    return out


_NC_CACHE = []


def kernel(x, w_qkv, w_out, b_out):
    """Full-input entry point: shards batch over 8 NeuronCores, runs the
    Bass kernel, gathers the full [64, 197, 768] fp32 output."""
    if not _NC_CACHE:
        _NC_CACHE.append(build_nc())
    nc = _NC_CACHE[0]
    in_maps = host_in_maps(
        np.asarray(x, dtype=np.float32),
        np.asarray(w_qkv, dtype=np.float32),
        np.asarray(w_out, dtype=np.float32),
        np.asarray(b_out, dtype=np.float32),
    )
    res = run_bass_kernel_spmd(nc, in_maps, core_ids=list(range(NCORES)))
    return host_gather(res.results)
